# revision 3
# baseline (speedup 1.0000x reference)
"""Trainium2 Bass kernel for CausalSelfAttention (RoPE + GQA), 8-core SPMD.

Sharding: 8 cores = 4 batches x 2 query-halves. Each core handles one batch
and four query-256-blocks paired {i, 7-i} so causal work is balanced; the
per-slot key-chunk counts are padded to (16, 12, 8, 4) so every core runs an
identical instruction stream -- all per-core variation (query gather order,
RoPE tables, causal masks) is input data.

Device pipeline per core:
  QKV projections (fp32r matmuls) -> RoPE on Q^T/K^T (NeoX-permuted weights
  make rotation halves contiguous) -> causal attention per (head, slot):
  S^T = K^T.T @ Q^T in fp32r, exp on ScalarE (PSUM->bf16, scale=1/8),
  bf16 mask multiply, P.V via bf16 matmul with a ones-augmented V column
  producing the softmax denominator for free -> divide -> output projection
  (fp32r). Host does the final gather/transpose.
"""
import sys

sys.path.insert(0, "/opt/trn_rl_repo")

import numpy as np
import ml_dtypes

B, T, C = 4, 2048, 576
H, HKV, D = 9, 3, 64
THETA = 10000.0
QB = 256                      # query block
TQ = 1024                     # queries per core
SLOT_PAD = [16, 12, 8, 4]     # padded key-chunk counts per slot
QBLOCKS = [[7, 5, 2, 0], [6, 4, 3, 1]]   # q-256-block ids per half j
CCX = [(0, 128), (128, 128), (256, 128), (384, 128), (512, 65)]   # x chunks (577 rows incl ones)
CCQ = [(0, 128), (128, 128), (256, 128), (384, 128), (512, 64)]   # 576-row chunks
MM = [(0, 128), (128, 128), (256, 128), (384, 128), (512, 64)]    # output-dim chunks of 576

_PROG = None


def _build_program():
    import concourse.bacc as bacc
    import concourse.mybir as mybir
    import concourse.tile as tile

    dt = mybir.dt
    f32, f32r, bf16 = dt.float32, dt.float32r, dt.bfloat16
    AF = mybir.ActivationFunctionType

    nc = bacc.Bacc("TRN2", target_bir_lowering=False, debug=False, num_devices=8)

    def inp(name, shape, d=f32):
        return nc.declare_dram_parameter(name, shape, d, isOutput=False)

    xkT = inp("xkT", [577, T])
    xqT = inp("xqT", [C, TQ])
    wqT = inp("wqT", [C, C])
    wkT = inp("wkT", [C, HKV * D])
    wvT = inp("wvT", [577, 260])
    woT = inp("woT", [C, C])
    c2k = inp("c2k", [128, T])
    s2k = inp("s2k", [128, T])
    c2q = inp("c2q", [128, TQ])
    s2q = inp("s2q", [128, TQ])
    masksp = inp("masks", [16 * 128, QB], bf16)
    yT = nc.declare_dram_parameter("yT", [C, TQ], f32, isOutput=True)

    with tile.TileContext(nc) as tc:
        with (
            tc.tile_pool(name="const", bufs=1) as cp,
            tc.tile_pool(name="stage", bufs=2) as stg,
            tc.tile_pool(name="rope", bufs=2) as rp,
            tc.tile_pool(name="pwork", bufs=3) as pw,
            tc.tile_pool(name="psum_s", bufs=2, space="PSUM") as psum_s,
            tc.tile_pool(name="psum_y", bufs=2, space="PSUM") as psum_y,
            tc.tile_pool(name="psum_p", bufs=2, space="PSUM") as psum_p,
        ):
            # ---------- weights: DMA f32 stage -> cast f32r ----------
            def load_cast_w(pool, param, chunks, cols, tag):
                tiles = []
                for i, (k0, kl) in enumerate(chunks):
                    t = pool.tile([128, cols], f32r, tag=f"{tag}{i}", name=f"{tag}{i}")
                    st = stg.tile([128, cols], f32, tag="wstage", name="wstage")
                    nc.sync.dma_start(st[:kl, :], param[k0:k0 + kl, :])
                    nc.vector.tensor_copy(t[:kl, :], st[:kl, :])
                    tiles.append(t)
                return tiles

            wo_r = load_cast_w(cp, woT, MM, C, "wo")

            # ---------- masks, ones ----------
            m_b = cp.tile([128, 16 * QB], bf16, tag="masks", name="masks")
            for i in range(16):
                nc.sync.dma_start(m_b[:, i * QB:(i + 1) * QB],
                                  masksp[i * 128:(i + 1) * 128, :])
            ones_f = cp.tile([1, D], f32, tag="ones_f", name="ones_f")
            nc.vector.memset(ones_f[:], 1.0)
            ones_r = cp.tile([1, D], f32r, tag="ones_r", name="ones_r")
            nc.vector.tensor_copy(ones_r[:], ones_f[:])

            # ---------- persistent outputs of projection phase ----------
            kt_h = [cp.tile([64, T], f32r, tag=f"kt{g}", name=f"kt{g}")
                    for g in range(HKV)]
            qth = [cp.tile([64, TQ], f32r, tag=f"qth{h}", name=f"qth{h}")
                   for h in range(H)]
            v_t = [cp.tile([128, 260], bf16, tag=f"v{c}", name=f"v{c}") for c in range(16)]
            ypr = [cp.tile([128, TQ], f32r, tag=f"ypr{p}", name=f"ypr{p}") for p in range(5)]

            def rope(ps, rows, cols0, n, c2t, s2t, dsts):
                """dsts[bi][:, cols0:cols0+n] = rope(ps[64*bi:64*bi+64, 0:n])"""
                qsw = rp.tile([128, 512], f32, tag="ropesw", name="ropesw")
                for h0 in range(0, rows, 64):
                    nc.vector.tensor_copy(qsw[h0:h0 + 32, :n], ps[h0 + 32:h0 + 64, :n])
                    nc.vector.tensor_copy(qsw[h0 + 32:h0 + 64, :n], ps[h0:h0 + 32, :n])
                t1 = rp.tile([128, 512], f32r, tag="rope1", name="rope1")
                t2 = rp.tile([128, 512], f32r, tag="rope2", name="rope2")
                nc.vector.tensor_mul(t1[:rows, :n], ps[:rows, :n],
                                     c2t[:rows, cols0:cols0 + n])
                nc.gpsimd.tensor_mul(t2[:rows, :n], qsw[:rows, :n],
                                     s2t[:rows, cols0:cols0 + n])
                for bi, dt_ in enumerate(dsts):
                    nc.gpsimd.tensor_add(dt_[0:64, cols0:cols0 + n],
                                         t1[64 * bi:64 * bi + 64, :n],
                                         t2[64 * bi:64 * bi + 64, :n])

            # ---------- phase 1: xk load/cast, K-proj+rope, V-proj ----------
            with tc.tile_pool(name="wkv", bufs=1) as wkvp:
                wk_r = load_cast_w(wkvp, wkT, CCQ, HKV * D, "wk")
                wv_r = load_cast_w(wkvp, wvT, CCX, 260, "wv")
                c2k_t = wkvp.tile([128, T], f32, tag="c2k", name="c2k")
                s2k_t = wkvp.tile([128, T], f32, tag="s2k", name="s2k")
                nc.sync.dma_start(c2k_t[:], c2k[:])
                nc.sync.dma_start(s2k_t[:], s2k[:])
                with tc.tile_pool(name="xk", bufs=2) as xkp:
                    for nn_ in range(4):
                        xk_r = []
                        for i, (k0, kl) in enumerate(CCX):
                            t = xkp.tile([128, 512], f32r, tag=f"xk{i}",
                                         name=f"xk{i}")
                            st = stg.tile([128, 512], f32, tag="xstage",
                                          name="xstage")
                            nc.sync.dma_start(
                                st[:kl, :],
                                xkT[k0:k0 + kl, 512 * nn_:512 * (nn_ + 1)])
                            nc.gpsimd.tensor_copy(t[:kl, :], st[:kl, :])
                            xk_r.append(t)

                        # K-proj for this 512-key window
                        for mi, (mc0, mrows) in enumerate([(0, 128), (128, 64)]):
                            ps = psum_p.tile([128, 512], f32, tag="proj",
                                             name="proj")
                            for ci, (k0, kl) in enumerate(CCQ):
                                nc.tensor.matmul(
                                    ps[:mrows, :],
                                    wk_r[ci][:kl, mc0:mc0 + mrows],
                                    xk_r[ci][:kl, :],
                                    start=(ci == 0), stop=(ci == 4))
                            rope(ps, mrows, 512 * nn_, 512, c2k_t, s2k_t,
                                 [kt_h[0], kt_h[1]] if mi == 0 else [kt_h[2]])

                        # V-proj for the 4 key t-chunks in this window
                        for ti in range(4):
                            t_ = 4 * nn_ + ti
                            ps = psum_p.tile([128, 512], f32, tag="proj",
                                             name="proj")
                            for ci, (k0, kl) in enumerate(CCX):
                                nc.tensor.matmul(
                                    ps[:, :260],
                                    xk_r[ci][:kl, 128 * ti:128 * (ti + 1)],
                                    wv_r[ci][:kl, :],
                                    start=(ci == 0), stop=(ci == 4))
                            nc.scalar.activation(v_t[t_][:], ps[:, :260], AF.Copy)

            # ---------- phase 2: xq load/cast, Q-proj+rope ----------
            with tc.tile_pool(name="wq", bufs=1) as wqp:
                wq_r = load_cast_w(wqp, wqT, CCQ, C, "wq")
                c2q_t = wqp.tile([128, TQ], f32, tag="c2q", name="c2q")
                s2q_t = wqp.tile([128, TQ], f32, tag="s2q", name="s2q")
                nc.sync.dma_start(c2q_t[:], c2q[:])
                nc.sync.dma_start(s2q_t[:], s2q[:])
                with tc.tile_pool(name="xq", bufs=2) as xqp:
                    for nn_ in range(2):
                        xq_r = []
                        for i, (k0, kl) in enumerate(CCQ):
                            t = xqp.tile([128, 512], f32r, tag=f"xq{i}",
                                         name=f"xq{i}")
                            st = stg.tile([128, 512], f32, tag="xstage",
                                          name="xstage")
                            nc.sync.dma_start(
                                st[:kl, :],
                                xqT[k0:k0 + kl, 512 * nn_:512 * (nn_ + 1)])
                            nc.gpsimd.tensor_copy(t[:kl, :], st[:kl, :])
                            xq_r.append(t)

                        for m, (mc0, mrows) in enumerate(MM):
                            ps = psum_p.tile([128, 512], f32, tag="proj",
                                             name="proj")
                            for ci, (k0, kl) in enumerate(CCQ):
                                nc.tensor.matmul(
                                    ps[:mrows, :],
                                    wq_r[ci][:kl, mc0:mc0 + mrows],
                                    xq_r[ci][:kl, :],
                                    start=(ci == 0), stop=(ci == 4))
                            dsts = ([qth[2 * m], qth[2 * m + 1]] if m < 4
                                    else [qth[8]])
                            rope(ps, mrows, 512 * nn_, 512, c2q_t, s2q_t, dsts)

            # ---------- phase 3: attention ----------
            for s in range(4):
                n = SLOT_PAD[s]
                for h in range(H):
                    g = h // 3
                    hp, hr = h // 2, 64 * (h % 2)
                    y_ps = psum_y.tile([65, QB], f32, tag="ypsum", name="ypsum")
                    for sc in range(n // 4):
                        sp = psum_s.tile([128, 4 * QB], f32, tag="scores", name="scores")
                        for i in range(4):
                            c = 4 * sc + i
                            nc.tensor.matmul(
                                sp[:, QB * i:QB * (i + 1)],
                                kt_h[g][0:64, 128 * c:128 * (c + 1)],
                                qth[h][0:64, QB * s:QB * (s + 1)],
                                start=True, stop=True)
                        p_b = pw.tile([128, 4 * QB], bf16, tag="p", name="p")
                        nc.scalar.activation(p_b[:], sp[:], AF.Exp, scale=0.125)
                        if sc == n // 4 - 1:
                            nc.vector.tensor_mul(
                                p_b[:], p_b[:],
                                m_b[:, 1024 * s:1024 * (s + 1)])
                        for i in range(4):
                            c = 4 * sc + i
                            nc.tensor.matmul(
                                y_ps[:], v_t[c][:, 65 * g:65 * g + 65],
                                p_b[:, QB * i:QB * (i + 1)],
                                start=(c == 0), stop=(c == n - 1))
                    # divide by denominator (row 64)
                    recip = pw.tile([1, QB], f32r, tag="recip", name="recip")
                    with nc.allow_low_precision(reason="f32r softmax denom"):
                        nc.vector.reciprocal(recip[:], y_ps[64:65, :])
                    rb_ps = psum_p.tile([128, 512], f32, tag="proj", name="proj")
                    nc.tensor.matmul(rb_ps[:D, :QB], ones_r[:], recip[:],
                                     start=True, stop=True)
                    rb_sb = pw.tile([D, QB], f32, tag="rb", name="rb")
                    nc.vector.tensor_copy(rb_sb[:], rb_ps[:D, :QB])
                    nc.vector.tensor_mul(
                        ypr[hp][hr:hr + 64, QB * s:QB * (s + 1)],
                        y_ps[0:64, :], rb_sb[:])

            # ---------- phase 4: output projection ----------
            for nn_ in range(2):
                for m, (mc0, mrows) in enumerate(MM):
                    ps = psum_p.tile([128, 512], f32, tag="proj", name="proj")
                    for p, (pc0, pl) in enumerate(MM):
                        nc.tensor.matmul(
                            ps[:mrows, :],
                            wo_r[p][:pl, mc0:mc0 + mrows],
                            ypr[p][:pl, 512 * nn_:512 * (nn_ + 1)],
                            start=(p == 0), stop=(p == 4))
                    ost = pw.tile([128, 512], f32, tag="ostage", name="ostage")
                    nc.scalar.activation(ost[:mrows, :], ps[:mrows, :], AF.Copy)
                    nc.sync.dma_start(
                        yT[mc0:mc0 + mrows, 512 * nn_:512 * (nn_ + 1)],
                        ost[:mrows, :])

    nc.compile()
    return nc


def _get_program():
    global _PROG
    if _PROG is None:
        _PROG = _build_program()
    return _PROG


def _host_prep(x, Wq, Wk, Wv, Wo):
    """Build the 8 per-core input maps."""
    # NeoX deinterleave permutation within each 64-wide head
    def neox_perm(nheads):
        p = []
        for h in range(nheads):
            p += [64 * h + 2 * j for j in range(32)]
            p += [64 * h + 2 * j + 1 for j in range(32)]
        return np.array(p)

    permq = neox_perm(H)
    permk = neox_perm(HKV)
    wqT = np.ascontiguousarray(Wq[permq].T)           # [576, 576]
    wkT = np.ascontiguousarray(Wk[permk].T)           # [576, 192]
    woT = np.ascontiguousarray(Wo.T)                  # [576, 576]
    wvT = np.zeros((577, 260), np.float32)
    for g in range(HKV):
        wvT[:C, 65 * g:65 * g + 64] = Wv[64 * g:64 * g + 64].T
        wvT[576, 65 * g + 64] = 1.0                   # ones column via x's ones row

    invf = (THETA ** (-np.arange(32, dtype=np.float64) / 32)).astype(np.float64)

    def tables(pos):
        ang = pos[None, :] * invf[:, None]            # [32, n]
        cos, sin = np.cos(ang), np.sin(ang)
        c2 = np.tile(cos, (4, 1)).astype(np.float32)          # 128 rows
        s2 = np.tile(np.vstack([-sin, sin]), (2, 1)).astype(np.float32)
        return c2, s2

    c2k, s2k = tables(np.arange(T, dtype=np.float64))

    in_maps = []
    core_meta = []
    for b in range(B):
        for j in range(2):
            qblocks = QBLOCKS[j]
            qsel = np.concatenate(
                [np.arange(QB * q, QB * (q + 1)) for q in qblocks])
            xkT = np.vstack([x[b].T, np.ones((1, T), np.float32)])
            xqT = np.ascontiguousarray(x[b][qsel].T)
            c2q, s2q = tables(qsel.astype(np.float64))
            masks = np.zeros((16 * 128, QB), np.float32)
            for s in range(4):
                q = qblocks[s]
                for k in range(4):
                    c = SLOT_PAD[s] - 4 + k
                    ki = np.arange(128 * c, 128 * (c + 1))
                    qi = np.arange(QB * q, QB * (q + 1))
                    masks[(4 * s + k) * 128:(4 * s + k + 1) * 128] = (
                        ki[:, None] <= qi[None, :]).astype(np.float32)
            in_maps.append({
                "xkT": xkT, "xqT": xqT,
                "wqT": wqT, "wkT": wkT, "wvT": wvT, "woT": woT,
                "c2k": c2k, "s2k": s2k, "c2q": c2q, "s2q": s2q,
                "masks": masks.astype(ml_dtypes.bfloat16),
            })
            core_meta.append((b, qsel))
    return in_maps, core_meta


def kernel(x, Wq, Wk, Wv, Wo):
    x = np.asarray(x, np.float32)
    Wq = np.asarray(Wq, np.float32)
    Wk = np.asarray(Wk, np.float32)
    Wv = np.asarray(Wv, np.float32)
    Wo = np.asarray(Wo, np.float32)

    from concourse.bass_utils import run_bass_kernel_spmd

    nc = _get_program()
    in_maps, core_meta = _host_prep(x, Wq, Wk, Wv, Wo)
    res = run_bass_kernel_spmd(nc, in_maps, list(range(8)))

    out = np.empty((B, T, C), np.float32)
    for core, (b, qsel) in enumerate(core_meta):
        out[b, qsel, :] = res.results[core]["yT"].T
    return out
